# revision 1
# baseline (speedup 1.0000x reference)
"""Trainium2 Bass kernel for InteractorwoLSTM additive attention.

out[b,t,:] = alpha[b,t,:] @ h_s[b]  with
  beta[b,t,n] = W_w . tanh(h_s[b,n]@W_S + b_S + h_v[b,t]@W_V + b_V) + b_w
  alpha = masked-softmax(beta) per reference semantics.

Sharding: data-parallel over batch B=32 across 8 cores (4 batches/core);
all weights replicated.

Device layout (per core, per batch b):
  - D_I (=512) lives on partitions in 4 chunks of 128.
  - VT[c]  = (V[b]).T chunk      (128 d, 128 t)   via PE transpose + matmul
  - ST'[c] = (S[b]).T chunk + (b_S+b_V)  (128 d, 30 n)
  - e_pre  = VT broadcast-add ST'  (128, 30, 128)  on DVE (0-stride APs)
  - e      = tanh(e_pre)           on ACT
  - beta   = per-n matmuls lhsT=e[:,n,:], rhs=W_w chunk -> psum (128 t, 30 n)
  - masked softmax fused on DVE/ACT (exp accum_out gives Z; ttr gives Qsum)
  - alpha^T via PE transpose, final einsum = one matmul (K=30, N=512)
"""

import os
import numpy as np

B, T, N = 32, 128, 30
D = 512
NCORES = 8
BPC = B // NCORES  # batches per core
NC_CHUNKS = D // 128  # 4

_CACHE = {}


def _build(e_dtype_name: str, add_mode: str):
    import concourse.bacc as bacc
    import concourse.tile as tile
    from concourse import mybir
    import concourse.bass as bass
    from concourse.masks import make_identity

    f32 = mybir.dt.float32
    DT_E = getattr(mybir.dt, e_dtype_name)
    DT_VS = DT_E  # dtype of VT/ST tiles (bf16 enables DVE 4x tensor_scalar)

    nc = bacc.Bacc(
        "TRN2",
        target_bir_lowering=False,
        debug=False,
        enable_asserts=True,
        num_devices=NCORES,
    )

    # ---- DRAM I/O ----
    hs_d = nc.dram_tensor("h_s", [BPC, N, D], f32, kind="ExternalInput").ap()
    hv_d = nc.dram_tensor("h_v", [BPC, T, D], f32, kind="ExternalInput").ap()
    WS_d = nc.dram_tensor("W_S", [D, D], f32, kind="ExternalInput").ap()
    WV_d = nc.dram_tensor("W_V", [D, D], f32, kind="ExternalInput").ap()
    Ww_d = nc.dram_tensor("W_w", [D], f32, kind="ExternalInput").ap()
    bSV_d = nc.dram_tensor("bSV", [1, D], f32, kind="ExternalInput").ap()
    bw_d = nc.dram_tensor("b_w_rep", [128, 1], f32, kind="ExternalInput").ap()
    mask_d = nc.dram_tensor("mask_bc", [128, BPC, N], f32, kind="ExternalInput").ap()
    out_d = nc.dram_tensor("out", [BPC, T, D], f32, kind="ExternalOutput").ap()

    with tile.TileContext(nc) as tc:
        with (
            tc.tile_pool(name="const", bufs=1) as const,
            tc.tile_pool(name="hv", bufs=2) as hvp,
            tc.tile_pool(name="proj", bufs=2) as projp,
            tc.tile_pool(name="epre", bufs=2) as eprep,
            tc.tile_pool(name="ebig", bufs=2 if DT_E != f32 else 1) as ebigp,
            tc.tile_pool(name="soft", bufs=2) as softp,
            tc.tile_pool(name="pwork", bufs=3, space="PSUM") as pwork,
            tc.tile_pool(name="pbeta", bufs=2, space="PSUM") as pbeta,
            tc.tile_pool(name="pfin", bufs=2, space="PSUM") as pfin,
        ):
            # ---- constants / weights ----
            ident = const.tile([128, 128], f32)
            make_identity(nc, ident[:])

            WS_sb = const.tile([128, NC_CHUNKS, NC_CHUNKS, 128], f32)
            nc.sync.dma_start(
                out=WS_sb[:],
                in_=WS_d.rearrange("(kc p) (mc m) -> p kc mc m", p=128, m=128),
            )
            WV_sb = const.tile([128, NC_CHUNKS, NC_CHUNKS, 128], f32)
            nc.sync.dma_start(
                out=WV_sb[:],
                in_=WV_d.rearrange("(kc p) (mc m) -> p kc mc m", p=128, m=128),
            )
            Ww_sb = const.tile([128, NC_CHUNKS], DT_E)
            nc.sync.dma_start(out=Ww_sb[:], in_=Ww_d.rearrange("(c p) -> p c", p=128))
            bSV_sb = const.tile([1, D], f32)
            nc.sync.dma_start(out=bSV_sb[:], in_=bSV_d)
            bw_sb = const.tile([128, 1], f32)
            nc.sync.dma_start(out=bw_sb[:], in_=bw_d)
            mask_sb = const.tile([128, BPC, N], f32)
            nc.sync.dma_start(out=mask_sb[:], in_=mask_d)
            ones30 = const.tile([1, N], f32)
            nc.vector.memset(ones30[:], 1.0)
            hs_sb = const.tile([N, BPC, D], f32)
            for b in range(BPC):
                nc.sync.dma_start(out=hs_sb[:, b, :], in_=hs_d[b])

            for b in range(BPC):
                # ---- load + transpose h_v[b]; transpose h_s[b] ----
                hv_sb = hvp.tile([128, D], f32, tag="hv")
                nc.sync.dma_start(out=hv_sb[:], in_=hv_d[b])
                hvT = projp.tile([128, NC_CHUNKS, 128], f32, tag="hvT")
                hsT = projp.tile([128, NC_CHUNKS, N], f32, tag="hsT")
                for c in range(NC_CHUNKS):
                    ps = pwork.tile([128, 128], f32, tag="w")
                    nc.tensor.transpose(
                        ps[:, :128], hv_sb[:, c * 128 : (c + 1) * 128], ident[:]
                    )
                    nc.vector.tensor_copy(hvT[:, c, :], ps[:, :128])
                for c in range(NC_CHUNKS):
                    ps = pwork.tile([128, 128], f32, tag="w")
                    nc.tensor.transpose(
                        ps[:, :N],
                        hs_sb[:, b, c * 128 : (c + 1) * 128],
                        ident[:N, :N],
                    )
                    nc.vector.tensor_copy(hsT[:, c, :], ps[:, :N])

                # ---- projections: VT = (h_v W_V).T, ST' = (h_s W_S).T + bSV ----
                VT = projp.tile([128, NC_CHUNKS, 128], DT_VS, tag="VT")
                ST = projp.tile([128, NC_CHUNKS, N], DT_VS, tag="ST")
                for mc in range(NC_CHUNKS):
                    ps = pwork.tile([128, 128], f32, tag="w")
                    for kc in range(NC_CHUNKS):
                        nc.tensor.matmul(
                            ps[:, :128],
                            WV_sb[:, kc, mc, :],
                            hvT[:, kc, :],
                            start=(kc == 0),
                            stop=(kc == NC_CHUNKS - 1),
                        )
                    nc.vector.tensor_copy(VT[:, mc, :], ps[:, :128])
                for mc in range(NC_CHUNKS):
                    ps = pwork.tile([128, 128], f32, tag="w")
                    for kc in range(NC_CHUNKS):
                        nc.tensor.matmul(
                            ps[:, :N],
                            WS_sb[:, kc, mc, :],
                            hsT[:, kc, :],
                            start=(kc == 0),
                            stop=False,
                        )
                    nc.tensor.matmul(
                        ps[:, :N],
                        bSV_sb[0:1, mc * 128 : (mc + 1) * 128],
                        ones30[0:1, :],
                        start=False,
                        stop=True,
                    )
                    nc.vector.tensor_copy(ST[:, mc, :], ps[:, :N])

                # ---- e = tanh(VT (+bcast) ST') ; beta accumulation ----
                ebig = ebigp.tile([128, NC_CHUNKS, N, 128], DT_E, tag="e")
                beta_ps = pbeta.tile([128, N], f32, tag="beta")
                for c in range(NC_CHUNKS):
                    epre = eprep.tile([128, N, 128], DT_E, tag="epre")
                    if add_mode == "tt":
                        vt_b = VT[:, c, :].unsqueeze(1).broadcast_to([128, N, 128])
                        st_b = ST[:, c, :].unsqueeze(2).broadcast_to([128, N, 128])
                        nc.vector.tensor_add(epre[:], vt_b, st_b)
                    else:  # "ts": per-n tensor_scalar (per-partition scalar add)
                        for n in range(N):
                            nc.vector.tensor_scalar_add(
                                epre[:, n, :],
                                VT[:, c, :],
                                ST[:, c, n : n + 1],
                            )
                    nc.scalar.activation(
                        ebig[:, c, :, :],
                        epre[:],
                        mybir.ActivationFunctionType.Tanh,
                    )
                for n in range(N):
                    for c in range(NC_CHUNKS):
                        nc.tensor.matmul(
                            beta_ps[:, n : n + 1],
                            ebig[:, c, n, :],
                            Ww_sb[:, c : c + 1],
                            start=(c == 0),
                            stop=(c == NC_CHUNKS - 1),
                        )

                # ---- masked softmax (faithful to reference) ----
                m_b = mask_sb[:, b, :]
                q1 = softp.tile([128, N], f32, tag="q1")
                # q1 = (beta + b_w) * m
                nc.vector.tensor_scalar_add(q1[:], beta_ps[:], bw_sb[:])
                nc.vector.tensor_mul(q1[:], q1[:], m_b)
                t1 = softp.tile([128, N], f32, tag="t1")
                Z1 = softp.tile([128, 1], f32, tag="Z1")
                nc.scalar.activation(
                    t1[:], q1[:], mybir.ActivationFunctionType.Exp, accum_out=Z1[:]
                )
                q = softp.tile([128, N], f32, tag="q")
                Qs = softp.tile([128, 1], f32, tag="Qs")
                nc.vector.tensor_mul(q[:], t1[:], m_b)
                qc = softp.tile([128, N], f32, tag="qc")
                nc.scalar.activation(
                    qc[:], q[:], mybir.ActivationFunctionType.Copy, accum_out=Qs[:]
                )
                denom = softp.tile([128, 1], f32, tag="denom")
                nc.vector.tensor_scalar(
                    denom[:],
                    Z1[:],
                    1e-13,
                    Qs[:],
                    op0=mybir.AluOpType.mult,
                    op1=mybir.AluOpType.add,
                )
                recip = softp.tile([128, 1], f32, tag="recip")
                nc.vector.reciprocal(recip[:], denom[:])
                alpha = softp.tile([128, N], f32, tag="alpha")
                nc.vector.tensor_scalar(
                    alpha[:],
                    q[:],
                    recip[:],
                    1e-13,
                    op0=mybir.AluOpType.mult,
                    op1=mybir.AluOpType.add,
                )

                # ---- out[b] = alpha @ h_s[b] ----
                aT_ps = pfin.tile([N, 128], f32, tag="fin")
                nc.tensor.transpose(aT_ps[:], alpha[:], ident[:])
                aT = softp.tile([N, 128], f32, tag="aT")
                nc.vector.tensor_copy(aT[:], aT_ps[:])
                out_ps = pfin.tile([128, D], f32, tag="fin")
                nc.tensor.matmul(out_ps[:], aT[:], hs_sb[:, b, :], start=True, stop=True)
                out_sb = softp.tile([128, D], f32, tag="out")
                nc.vector.tensor_copy(out_sb[:], out_ps[:])
                nc.sync.dma_start(out=out_d[b], in_=out_sb[:])

    nc.compile()
    return nc


def _get_nc():
    e_dtype = os.environ.get("KERNEL_E_DTYPE", "float32")
    add_mode = os.environ.get("KERNEL_ADD_MODE", "tt")
    key = (e_dtype, add_mode)
    if key not in _CACHE:
        _CACHE[key] = _build(e_dtype, add_mode)
    return _CACHE[key]


def _make_in_maps(h_s, h_v, lengths, W_S, b_S, W_V, b_V, W_w, b_w):
    h_s = np.ascontiguousarray(h_s, dtype=np.float32)
    h_v = np.ascontiguousarray(h_v, dtype=np.float32)
    mask = (
        np.asarray(lengths).reshape(B, 1) >= np.arange(1, N + 1).reshape(1, N)
    ).astype(np.float32)
    WS = np.ascontiguousarray(W_S, dtype=np.float32)
    WV = np.ascontiguousarray(W_V, dtype=np.float32)
    Ww = np.ascontiguousarray(W_w, dtype=np.float32)
    bSV = np.ascontiguousarray((b_S + b_V).reshape(1, D), dtype=np.float32)
    bw_rep = np.full((128, 1), np.float32(np.asarray(b_w).reshape(-1)[0]))
    in_maps = []
    for c in range(NCORES):
        sl = slice(c * BPC, (c + 1) * BPC)
        mask_bc = np.ascontiguousarray(
            np.broadcast_to(mask[sl][None, :, :], (128, BPC, N)), dtype=np.float32
        )
        in_maps.append(
            {
                "h_s": h_s[sl],
                "h_v": h_v[sl],
                "W_S": WS,
                "W_V": WV,
                "W_w": Ww,
                "bSV": bSV,
                "b_w_rep": bw_rep,
                "mask_bc": mask_bc,
            }
        )
    return in_maps


def run(inputs: dict, trace: bool = False):
    """Run on 8 NeuronCores; returns (output, BassKernelResults)."""
    from concourse import bass_utils

    nc = _get_nc()
    in_maps = _make_in_maps(**inputs)
    res = bass_utils.run_bass_kernel_spmd(
        nc, in_maps, core_ids=list(range(NCORES)), trace=trace
    )
    outs = [r["out"] for r in res.results]
    full = np.concatenate(outs, axis=0).astype(np.float32)
    return full, res


def kernel(**inputs) -> np.ndarray:
    out, _ = run(inputs, trace=False)
    return out



# revision 11
# speedup vs baseline: 3.4679x; 3.4679x over previous
"""Trainium2 Bass kernel for InteractorwoLSTM additive attention.

out[b,t,:] = alpha[b,t,:] @ h_s[b]  with
  beta[b,t,n] = W_w . tanh(h_s[b,n]@W_S + b_S + h_v[b,t]@W_V + b_V) + b_w
  alpha = masked-softmax(beta) per reference semantics.

Sharding: data-parallel over batch B=32 across 8 cores (4 slots/core).

Key structural choices vs the naive version:
  - Masked n never contribute: q[n,t] = exp(beta*m)*m == 0 where m==0, and
    Z1's masked part is the constant count (30-L). So tanh/add/beta run
    only over n < L. The kernel is compiled per lengths-profile (cached);
    batches are bin-packed over cores so SPMD slot s has baked size
    N_slot[s] = max over cores of that slot's length.
  - All transposes (h_v^T, h_s^T) and tile packing happen on the host;
    every device DMA load is a single contiguous [P, X] transfer.
  - beta[n,t] = sum_d Ww[d]*e[d,n,t] via column-tiled matmuls: lhsT is the
    1-column Ww chunk at tile_position (0,32j), j=n%4, so 4 matmuls run
    concurrently in the PE array and beta lands multi-partition in PSUM.
  - Softmax collapses: Qs == Z1_mm (one ones-matmul), exp has no mask
    multiply, the alpha epsilon term (1e-13 * sum_n h_s) is added on host.
  - recip(denom) is folded into the final output copy as a per-partition
    tensor_scalar multiply.
"""

import numpy as np

B, T, N = 32, 128, 30
D = 512
NCORES = 8
BPC = B // NCORES  # batch slots per core
NC = D // 128  # 4 chunks of the feature dim

_CACHE = {}


def _build(n_slot: tuple[int, int, int, int]):
    import concourse.bacc as bacc
    import concourse.tile as tile
    from concourse import mybir

    f32 = mybir.dt.float32

    nc = bacc.Bacc(
        "TRN2",
        target_bir_lowering=False,
        debug=False,
        enable_asserts=True,
        num_devices=NCORES,
    )

    # ---- DRAM I/O (host pre-packed, all contiguous) ----
    hvT_d = nc.dram_tensor("hvT", [128, NC * BPC * T], f32, kind="ExternalInput").ap()
    hsT_d = nc.dram_tensor("hsT", [128, NC * BPC * N], f32, kind="ExternalInput").ap()
    hs_d = nc.dram_tensor("hs", [N, BPC * D], f32, kind="ExternalInput").ap()
    WV_d = nc.dram_tensor("WVp", [128, NC * NC * 128], f32, kind="ExternalInput").ap()
    WS_d = nc.dram_tensor("WSp", [128, NC * NC * 128], f32, kind="ExternalInput").ap()
    Ww_d = nc.dram_tensor("Wwp", [128, NC], f32, kind="ExternalInput").ap()
    bSV_d = nc.dram_tensor("bSVp", [128, NC], f32, kind="ExternalInput").ap()
    bw_d = nc.dram_tensor("bwp", [N, 1], f32, kind="ExternalInput").ap()
    mp_d = nc.dram_tensor("mp", [N, BPC], f32, kind="ExternalInput").ap()
    out_d = nc.dram_tensor("out", [BPC, T, D], f32, kind="ExternalOutput").ap()

    with tile.TileContext(nc) as tc:
        with (
            tc.tile_pool(name="const", bufs=1) as const,
            tc.tile_pool(name="proj", bufs=1) as projp,
            tc.tile_pool(name="epre", bufs=2) as eprep,
            tc.tile_pool(name="ebig", bufs=2) as ebigp,
            tc.tile_pool(name="soft", bufs=2) as softp,
            tc.tile_pool(name="pproj", bufs=2, space="PSUM") as pproj,
            tc.tile_pool(name="pbeta", bufs=1, space="PSUM") as pbeta,
            tc.tile_pool(name="pz", bufs=1, space="PSUM") as pzp,
            tc.tile_pool(name="pfin", bufs=2, space="PSUM") as pfin,
        ):
            # ---- constants / weights / activations (contiguous loads) ----
            WV_sb = const.tile([128, NC, NC, 128], f32)
            nc.sync.dma_start(out=WV_sb[:], in_=WV_d)
            WS_sb = const.tile([128, NC, NC, 128], f32)
            nc.sync.dma_start(out=WS_sb[:], in_=WS_d)
            Ww_sb = const.tile([128, NC], f32)
            nc.sync.dma_start(out=Ww_sb[:], in_=Ww_d)
            bSV_sb = const.tile([128, NC], f32)
            nc.sync.dma_start(out=bSV_sb[:], in_=bSV_d)
            bw_sb = const.tile([N, 1], f32)
            nc.sync.dma_start(out=bw_sb[:], in_=bw_d)
            mp_sb = const.tile([N, BPC], f32)
            nc.sync.dma_start(out=mp_sb[:], in_=mp_d)
            ones_sb = const.tile([N, 1], f32)
            nc.vector.memset(ones_sb[:], 1.0)
            z1_sb = const.tile([1, 128], f32)
            nc.vector.memset(z1_sb[:], 0.0)
            zrow_sb = const.tile([1, 512], f32)
            nc.vector.memset(zrow_sb[:], 0.0)

            hvT = const.tile([128, NC, BPC, T], f32)
            nc.sync.dma_start(out=hvT[:], in_=hvT_d)
            hsT = const.tile([128, NC, BPC, N], f32)
            nc.sync.dma_start(out=hsT[:], in_=hsT_d)
            hs_sb = const.tile([N, BPC, D], f32)
            nc.sync.dma_start(out=hs_sb[:], in_=hs_d)

            # ---- projections, all batch slots at once ----
            # VT[d, kc?, b, t]: psum[dmc, (b,t)] = sum_kc WV[kc,mc]^T-blk @ hvT
            VT = projp.tile([128, NC, BPC, T], f32)
            for mc in range(NC):
                ps = pproj.tile([128, BPC * T], f32, tag="pv")
                for kc in range(NC):
                    nc.tensor.matmul(
                        ps[:],
                        WV_sb[:, kc, mc, :],
                        hvT[:, kc, :, :],
                        start=(kc == 0),
                        stop=(kc == NC - 1),
                    )
                nc.vector.tensor_copy(VT[:, mc, :, :], ps[:])
            ST = projp.tile([128, NC, BPC, N], f32)
            for mc in range(NC):
                ps_t = pproj.tile([128, BPC * T], f32, tag="pv", name="ps_t")
                ps = ps_t[:, : BPC * N]
                for kc in range(NC):
                    nc.tensor.matmul(
                        ps[:],
                        WS_sb[:, kc, mc, :],
                        hsT[:, kc, :, :],
                        start=(kc == 0),
                        stop=(kc == NC - 1),
                    )
                # fold b_S + b_V while copying out of PSUM
                nc.vector.tensor_scalar_add(
                    ST[:, mc, :, :], ps[:], bSV_sb[:, mc : mc + 1]
                )

            for b in range(BPC):
                L = n_slot[b]
                nr = (L + 3) // 4  # beta column-tile rounds
                # ---- e = tanh(VT (+bcast) ST), chunk by chunk; beta ----
                pbA = pbeta.tile([128, 512], f32, tag="bA", name="pbA")
                pbB = (
                    pbeta.tile([128, 512], f32, tag="bB", name="pbB")
                    if nr > 4
                    else None
                )
                # start=True clears the WHOLE 2KB bank's has-written bits, so
                # exactly one start per bank: a full-bank zeroing matmul whose
                # [128,512] write also forces WAW ordering before every
                # accumulate-only beta matmul below.
                for pb in (pbA, pbB):
                    if pb is not None:
                        nc.tensor.matmul(
                            pb[:],
                            z1_sb[:],
                            zrow_sb[:],
                            start=True,
                            stop=True,
                            skip_group_check=True,
                        )
                for c in range(NC):
                    epre_t = eprep.tile([128, N, T], f32, tag="epre")
                    epre = epre_t[:, :L, :]
                    vt_b = VT[:, c, b, :].unsqueeze(1).broadcast_to([128, L, T])
                    st_b = ST[:, c, b, :L].unsqueeze(2).broadcast_to([128, L, T])
                    nc.vector.tensor_add(epre, vt_b, st_b)
                    e_t = ebigp.tile([128, N, T], f32, tag="e")
                    e_c = e_t[:, :L, :]
                    nc.scalar.activation(
                        e_c, epre, mybir.ActivationFunctionType.Tanh
                    )
                    # beta: per (bank, col-group j) one strided-rhs matmul
                    # accumulating Ww_c . e_c over r-slots; n = 4r + j.
                    for base, pb in ((0, pbA), (16, pbB)):
                        if pb is None:
                            continue
                        hi = min(L, base + 16)
                        for j in range(4):
                            nj = len(range(base + j, hi, 4))
                            if nj == 0:
                                continue
                            nc.tensor.matmul(
                                pb[32 * j : 32 * j + 1, 0 : nj * T],
                                Ww_sb[:, c : c + 1],
                                e_c[:, base + j : hi : 4, :],
                                start=False,
                                stop=False,
                                tile_position=(0, 32 * j),
                                skip_group_check=True,
                            )

                # ---- beta [4j, r, t] -> [n, t] via SBUF->SBUF DMA ----
                bsb = softp.tile([128, 8, T], f32, tag="bsb")
                nc.vector.tensor_copy(bsb[:, 0:4, :], pbA[:].rearrange("p (r t) -> p r t", r=4))
                if pbB is not None:
                    nc.vector.tensor_copy(
                        bsb[:, 4:8, :], pbB[:].rearrange("p (r t) -> p r t", r=4)
                    )
                beta_nt = softp.tile([N, T], f32, tag="bnt")
                for j in range(min(4, L)):
                    nj = (L - 1 - j) // 4 + 1  # count of n in {j, j+4, ...} < L
                    nc.sync.dma_start(
                        out=beta_nt[j : j + 4 * (nj - 1) + 1 : 4, :],
                        in_=bsb[32 * j : 32 * j + 1, 0:nj, :],
                    )

                # ---- collapsed softmax ----
                # t1 = exp(beta + b_w);  Z = sum_{n<L} t1 (ones-matmul)
                t1 = softp.tile([N, T], f32, tag="t1")
                nc.scalar.activation(
                    t1[:L, :],
                    beta_nt[:L, :],
                    mybir.ActivationFunctionType.Exp,
                    bias=bw_sb[:L, :],
                )
                # per-core mask within the slot: q = t1 * m  (m=0 for n>=L_b)
                q_t = softp.tile([N, T], f32, tag="q")
                nc.vector.tensor_scalar(
                    q_t[:L, :],
                    t1[:L, :],
                    mp_sb[:L, b : b + 1],
                    None,
                    op0=mybir.AluOpType.mult,
                )
                pz = pzp.tile([128, 1], f32, tag="pz")
                nc.tensor.matmul(
                    pz[:, 0:1], q_t[:L, :], ones_sb[:L, :], start=True, stop=True
                )
                recip = softp.tile([128, 1], f32, tag="recip")
                nc.vector.reciprocal(recip[:], pz[:, 0:1])

                # ---- out[t,:] = recip[t] * sum_{n<L} t1[n,t] * h_s[n,:] ----
                pf = pfin.tile([128, D], f32, tag="fin")
                nc.tensor.matmul(
                    pf[:], q_t[:L, :], hs_sb[:L, b, :], start=True, stop=True
                )
                out_sb = softp.tile([128, D], f32, tag="out")
                nc.vector.tensor_scalar(
                    out_sb[:],
                    pf[:],
                    recip[:],
                    None,
                    op0=mybir.AluOpType.mult,
                )
                nc.sync.dma_start(out=out_d[b], in_=out_sb[:])

    nc.compile()
    return nc


def _get_nc(n_slot):
    key = tuple(n_slot)
    if key not in _CACHE:
        _CACHE[key] = _build(key)
    return _CACHE[key]


def _plan(lengths):
    """Assign 32 batches to 8 cores x 4 slots, slot s holding the s-th
    longest of each core; returns (assign[core][slot] = batch idx, n_slot)."""
    order = np.argsort(-lengths, kind="stable")
    assign = [[int(order[s * NCORES + c]) for s in range(BPC)] for c in range(NCORES)]
    n_slot = []
    for s in range(BPC):
        m = max(int(lengths[assign[c][s]]) for c in range(NCORES))
        n_slot.append(m)
    return assign, tuple(n_slot)


def _make_in_maps(h_s, h_v, lengths, W_S, b_S, W_V, b_V, W_w, b_w, assign):
    h_s = np.ascontiguousarray(h_s, dtype=np.float32)
    h_v = np.ascontiguousarray(h_v, dtype=np.float32)
    WS = np.asarray(W_S, dtype=np.float32)
    WV = np.asarray(W_V, dtype=np.float32)
    # [k, d] -> [p, kc, mc, 128] with k = kc*128+p
    WSp = np.ascontiguousarray(
        WS.reshape(NC, 128, NC, 128).transpose(1, 0, 2, 3)
    ).reshape(128, -1)
    WVp = np.ascontiguousarray(
        WV.reshape(NC, 128, NC, 128).transpose(1, 0, 2, 3)
    ).reshape(128, -1)
    Wwp = np.ascontiguousarray(np.asarray(W_w, dtype=np.float32).reshape(NC, 128).T)
    bSVp = np.ascontiguousarray(
        (np.asarray(b_S) + np.asarray(b_V)).astype(np.float32).reshape(NC, 128).T
    )
    bwp = np.full((N, 1), np.float32(np.asarray(b_w).reshape(-1)[0]))

    in_maps = []
    for c in range(NCORES):
        bidx = assign[c]
        hv_c = h_v[bidx]  # [BPC, T, D]
        hs_c = h_s[bidx]  # [BPC, N, D]
        # hvT: [p, kc, b, t] with d = kc*128+p
        hvT = np.ascontiguousarray(
            hv_c.transpose(2, 0, 1).reshape(NC, 128, BPC, T).transpose(1, 0, 2, 3)
        ).reshape(128, -1)
        hsT = np.ascontiguousarray(
            hs_c.transpose(2, 0, 1).reshape(NC, 128, BPC, N).transpose(1, 0, 2, 3)
        ).reshape(128, -1)
        hs_nat = np.ascontiguousarray(hs_c.transpose(1, 0, 2)).reshape(N, -1)
        mp = np.zeros((N, BPC), dtype=np.float32)
        for s in range(BPC):
            mp[: int(lengths[bidx[s]]), s] = 1.0
        in_maps.append(
            {
                "mp": mp,
                "hvT": hvT,
                "hsT": hsT,
                "hs": hs_nat,
                "WVp": WVp,
                "WSp": WSp,
                "Wwp": Wwp,
                "bSVp": bSVp,
                "bwp": bwp,
            }
        )
    return in_maps


def run(inputs: dict, trace: bool = False):
    """Run on 8 NeuronCores; returns (output, BassKernelResults)."""
    from concourse import bass_utils

    lengths = np.asarray(inputs["lengths"]).astype(np.int64)
    assign, n_slot = _plan(lengths)
    nc = _get_nc(n_slot)
    in_maps = _make_in_maps(
        inputs["h_s"], inputs["h_v"], lengths,
        inputs["W_S"], inputs["b_S"], inputs["W_V"], inputs["b_V"],
        inputs["W_w"], inputs["b_w"], assign,
    )
    res = bass_utils.run_bass_kernel_spmd(
        nc, in_maps, core_ids=list(range(NCORES)), trace=trace
    )
    h_s = np.asarray(inputs["h_s"], dtype=np.float32)
    full = np.empty((B, T, D), dtype=np.float32)
    for c in range(NCORES):
        out_c = res.results[c]["out"]
        for s in range(BPC):
            full[assign[c][s]] = out_c[s]
    # alpha's +1e-13 epsilon term, applied on host
    full += 1e-13 * h_s.sum(axis=1, keepdims=True)
    return full, res


def kernel(**inputs) -> np.ndarray:
    out, _ = run(inputs, trace=False)
    return out


# revision 12
# speedup vs baseline: 5.0743x; 1.4632x over previous
"""Trainium2 Bass kernel for InteractorwoLSTM additive attention.

out[b,t,:] = alpha[b,t,:] @ h_s[b]  with
  beta[b,t,n] = W_w . tanh(h_s[b,n]@W_S + b_S + h_v[b,t]@W_V + b_V) + b_w
  alpha = masked-softmax(beta) per reference semantics.

Sharding: data-parallel over batch B=32 across 8 cores (4 slots/core).

Key structural choices vs the naive version:
  - Masked n never contribute: q[n,t] = exp(beta*m)*m == 0 where m==0, and
    Z1's masked part is the constant count (30-L). So tanh/add/beta run
    only over n < L. The kernel is compiled per lengths-profile (cached);
    batches are bin-packed over cores so SPMD slot s has baked size
    N_slot[s] = max over cores of that slot's length.
  - All transposes (h_v^T, h_s^T) and tile packing happen on the host;
    every device DMA load is a single contiguous [P, X] transfer.
  - beta[n,t] = sum_d Ww[d]*e[d,n,t] via column-tiled matmuls: lhsT is the
    1-column Ww chunk at tile_position (0,32j), j=n%4, so 4 matmuls run
    concurrently in the PE array and beta lands multi-partition in PSUM.
  - Softmax collapses: Qs == Z1_mm (one ones-matmul), exp has no mask
    multiply, the alpha epsilon term (1e-13 * sum_n h_s) is added on host.
  - recip(denom) is folded into the final output copy as a per-partition
    tensor_scalar multiply.
"""

import ml_dtypes
import numpy as np

BF16 = ml_dtypes.bfloat16

B, T, N = 32, 128, 30
D = 512
NCORES = 8
BPC = B // NCORES  # batch slots per core
NC = D // 128  # 4 chunks of the feature dim

_CACHE = {}


def _build(n_slot: tuple[int, int, int, int]):
    import concourse.bacc as bacc
    import concourse.tile as tile
    from concourse import mybir

    f32 = mybir.dt.float32
    bf16 = mybir.dt.bfloat16

    nc = bacc.Bacc(
        "TRN2",
        target_bir_lowering=False,
        debug=False,
        enable_asserts=True,
        num_devices=NCORES,
    )

    # ---- DRAM I/O (host pre-packed, all contiguous) ----
    hvT_d = nc.dram_tensor("hvT", [128, NC * BPC * T], bf16, kind="ExternalInput").ap()
    hsT_d = nc.dram_tensor("hsT", [128, NC * BPC * N], bf16, kind="ExternalInput").ap()
    hs_d = nc.dram_tensor("hs", [N, BPC * D], f32, kind="ExternalInput").ap()
    WV_d = nc.dram_tensor("WVp", [128, NC * NC * 128], bf16, kind="ExternalInput").ap()
    WS_d = nc.dram_tensor("WSp", [128, NC * NC * 128], bf16, kind="ExternalInput").ap()
    Ww_d = nc.dram_tensor("Wwp", [128, NC], bf16, kind="ExternalInput").ap()
    bSV_d = nc.dram_tensor("bSVp", [128, NC], f32, kind="ExternalInput").ap()
    bw_d = nc.dram_tensor("bwp", [N, 1], f32, kind="ExternalInput").ap()
    mp_d = nc.dram_tensor("mp", [N, BPC], f32, kind="ExternalInput").ap()
    out_d = nc.dram_tensor("out", [BPC, T, D], f32, kind="ExternalOutput").ap()

    with tile.TileContext(nc) as tc:
        with (
            tc.tile_pool(name="const", bufs=1) as const,
            tc.tile_pool(name="proj", bufs=1) as projp,
            tc.tile_pool(name="epre", bufs=2) as eprep,
            tc.tile_pool(name="ebig", bufs=2) as ebigp,
            tc.tile_pool(name="soft", bufs=2) as softp,
            tc.tile_pool(name="pproj", bufs=2, space="PSUM") as pproj,
            tc.tile_pool(name="pbeta", bufs=1, space="PSUM") as pbeta,
            tc.tile_pool(name="pz", bufs=1, space="PSUM") as pzp,
            tc.tile_pool(name="pfin", bufs=2, space="PSUM") as pfin,
        ):
            # ---- constants / weights / activations (contiguous loads) ----
            WV_sb = const.tile([128, NC, NC, 128], bf16)
            nc.sync.dma_start(out=WV_sb[:], in_=WV_d)
            WS_sb = const.tile([128, NC, NC, 128], bf16)
            nc.sync.dma_start(out=WS_sb[:], in_=WS_d)
            Ww_sb = const.tile([128, NC], bf16)
            nc.sync.dma_start(out=Ww_sb[:], in_=Ww_d)
            bSV_sb = const.tile([128, NC], f32)
            nc.sync.dma_start(out=bSV_sb[:], in_=bSV_d)
            bw_sb = const.tile([N, 1], f32)
            nc.sync.dma_start(out=bw_sb[:], in_=bw_d)
            mp_sb = const.tile([N, BPC], f32)
            nc.sync.dma_start(out=mp_sb[:], in_=mp_d)
            ones_sb = const.tile([N, 1], f32)
            nc.vector.memset(ones_sb[:], 1.0)
            z1_sb = const.tile([1, 128], f32)
            nc.vector.memset(z1_sb[:], 0.0)
            zrow_sb = const.tile([1, 512], f32)
            nc.vector.memset(zrow_sb[:], 0.0)

            hvT = const.tile([128, NC, BPC, T], bf16)
            nc.sync.dma_start(out=hvT[:], in_=hvT_d)
            hsT = const.tile([128, NC, BPC, N], bf16)
            nc.sync.dma_start(out=hsT[:], in_=hsT_d)
            hs_sb = const.tile([N, BPC, D], f32)
            nc.sync.dma_start(out=hs_sb[:], in_=hs_d)

            # ---- projections, all batch slots at once ----
            # VT[d, kc?, b, t]: psum[dmc, (b,t)] = sum_kc WV[kc,mc]^T-blk @ hvT
            VT = projp.tile([128, NC, BPC, T], bf16)
            for mc in range(NC):
                ps = pproj.tile([128, BPC * T], f32, tag="pv")
                for kc in range(NC):
                    nc.tensor.matmul(
                        ps[:],
                        WV_sb[:, kc, mc, :],
                        hvT[:, kc, :, :],
                        start=(kc == 0),
                        stop=(kc == NC - 1),
                    )
                nc.vector.tensor_copy(VT[:, mc, :, :], ps[:])
            ST = projp.tile([128, NC, BPC, N, 2], bf16)
            for mc in range(NC):
                ps_t = pproj.tile([128, BPC * T], f32, tag="pv", name="ps_t")
                ps = ps_t[:, : BPC * N]
                for kc in range(NC):
                    nc.tensor.matmul(
                        ps[:],
                        WS_sb[:, kc, mc, :],
                        hsT[:, kc, :, :],
                        start=(kc == 0),
                        stop=(kc == NC - 1),
                    )
                # fold b_S + b_V while copying out of PSUM; write each value
                # twice (pair-doubled layout) so the epre add's last AP dim is
                # step-1 x2 on every operand -> DVE 2x_1P mode
                nc.vector.tensor_scalar_add(
                    ST[:, mc, :, :, :],
                    ps[:].unsqueeze(2).broadcast_to([128, BPC * N, 2]).rearrange(
                        "p (b n) two -> p b n two", b=BPC
                    ),
                    bSV_sb[:, mc : mc + 1],
                )

            for b in range(BPC):
                L = n_slot[b]
                nr = (L + 3) // 4  # beta column-tile rounds
                # ---- e = tanh(VT (+bcast) ST), chunk by chunk; beta ----
                pbA = pbeta.tile([128, 512], f32, tag="bA", name="pbA")
                pbB = (
                    pbeta.tile([128, 512], f32, tag="bB", name="pbB")
                    if nr > 4
                    else None
                )
                # start=True clears the WHOLE 2KB bank's has-written bits, so
                # exactly one start per bank: a full-bank zeroing matmul whose
                # [128,512] write also forces WAW ordering before every
                # accumulate-only beta matmul below.
                for pb in (pbA, pbB):
                    if pb is not None:
                        nc.tensor.matmul(
                            pb[:],
                            z1_sb[:],
                            zrow_sb[:],
                            start=True,
                            stop=True,
                            skip_group_check=True,
                        )
                for c in range(NC):
                    epre_t = eprep.tile([128, N, T], bf16, tag="epre")
                    epre = epre_t[:, :L, :]
                    vt_b = (
                        VT[:, c, b, :]
                        .rearrange("p (a two) -> p a two", two=2)
                        .unsqueeze(1)
                        .broadcast_to([128, L, T // 2, 2])
                    )
                    st_b = (
                        ST[:, c, b, :L, :]
                        .unsqueeze(2)
                        .broadcast_to([128, L, T // 2, 2])
                    )
                    nc.vector.tensor_add(
                        epre.rearrange("p l (a two) -> p l a two", two=2),
                        vt_b,
                        st_b,
                    )
                    e_t = ebigp.tile([128, N, T], bf16, tag="e")
                    e_c = e_t[:, :L, :]
                    nc.scalar.activation(
                        e_c, epre, mybir.ActivationFunctionType.Tanh
                    )
                    # beta: per (bank, col-group j) one strided-rhs matmul
                    # accumulating Ww_c . e_c over r-slots; n = 4r + j.
                    for base, pb in ((0, pbA), (16, pbB)):
                        if pb is None:
                            continue
                        hi = min(L, base + 16)
                        for j in range(4):
                            nj = len(range(base + j, hi, 4))
                            if nj == 0:
                                continue
                            nc.tensor.matmul(
                                pb[32 * j : 32 * j + 1, 0 : nj * T],
                                Ww_sb[:, c : c + 1],
                                e_c[:, base + j : hi : 4, :],
                                start=False,
                                stop=False,
                                tile_position=(0, 32 * j),
                                skip_group_check=True,
                            )

                # ---- beta [4j, r, t] -> [n, t] via SBUF->SBUF DMA ----
                bsb = softp.tile([128, 8, T], f32, tag="bsb")
                nc.vector.tensor_copy(bsb[:, 0:4, :], pbA[:].rearrange("p (r t) -> p r t", r=4))
                if pbB is not None:
                    nc.vector.tensor_copy(
                        bsb[:, 4:8, :], pbB[:].rearrange("p (r t) -> p r t", r=4)
                    )
                beta_nt = softp.tile([N, T], f32, tag="bnt")
                for j in range(min(4, L)):
                    nj = (L - 1 - j) // 4 + 1  # count of n in {j, j+4, ...} < L
                    nc.sync.dma_start(
                        out=beta_nt[j : j + 4 * (nj - 1) + 1 : 4, :],
                        in_=bsb[32 * j : 32 * j + 1, 0:nj, :],
                    )

                # ---- collapsed softmax ----
                # t1 = exp(beta + b_w);  Z = sum_{n<L} t1 (ones-matmul)
                t1 = softp.tile([N, T], f32, tag="t1")
                nc.scalar.activation(
                    t1[:L, :],
                    beta_nt[:L, :],
                    mybir.ActivationFunctionType.Exp,
                    bias=bw_sb[:L, :],
                )
                # per-core mask within the slot: q = t1 * m  (m=0 for n>=L_b)
                q_t = softp.tile([N, T], f32, tag="q")
                nc.vector.tensor_scalar(
                    q_t[:L, :],
                    t1[:L, :],
                    mp_sb[:L, b : b + 1],
                    None,
                    op0=mybir.AluOpType.mult,
                )
                pz = pzp.tile([128, 1], f32, tag="pz")
                nc.tensor.matmul(
                    pz[:, 0:1], q_t[:L, :], ones_sb[:L, :], start=True, stop=True
                )
                recip = softp.tile([128, 1], f32, tag="recip")
                nc.vector.reciprocal(recip[:], pz[:, 0:1])

                # ---- out[t,:] = recip[t] * sum_{n<L} t1[n,t] * h_s[n,:] ----
                pf = pfin.tile([128, D], f32, tag="fin")
                nc.tensor.matmul(
                    pf[:], q_t[:L, :], hs_sb[:L, b, :], start=True, stop=True
                )
                out_sb = softp.tile([128, D], f32, tag="out")
                nc.vector.tensor_scalar(
                    out_sb[:],
                    pf[:],
                    recip[:],
                    None,
                    op0=mybir.AluOpType.mult,
                )
                nc.sync.dma_start(out=out_d[b], in_=out_sb[:])

    nc.compile()
    return nc


def _get_nc(n_slot):
    key = tuple(n_slot)
    if key not in _CACHE:
        _CACHE[key] = _build(key)
    return _CACHE[key]


def _plan(lengths):
    """Assign 32 batches to 8 cores x 4 slots, slot s holding the s-th
    longest of each core; returns (assign[core][slot] = batch idx, n_slot)."""
    order = np.argsort(-lengths, kind="stable")
    assign = [[int(order[s * NCORES + c]) for s in range(BPC)] for c in range(NCORES)]
    n_slot = []
    for s in range(BPC):
        m = max(int(lengths[assign[c][s]]) for c in range(NCORES))
        n_slot.append(m)
    return assign, tuple(n_slot)


def _make_in_maps(h_s, h_v, lengths, W_S, b_S, W_V, b_V, W_w, b_w, assign):
    h_s = np.ascontiguousarray(h_s, dtype=np.float32)
    h_v = np.ascontiguousarray(h_v, dtype=np.float32)
    WS = np.asarray(W_S, dtype=np.float32)
    WV = np.asarray(W_V, dtype=np.float32)
    # [k, d] -> [p, kc, mc, 128] with k = kc*128+p
    WSp = np.ascontiguousarray(
        WS.reshape(NC, 128, NC, 128).transpose(1, 0, 2, 3)
    ).reshape(128, -1).astype(BF16)
    WVp = np.ascontiguousarray(
        WV.reshape(NC, 128, NC, 128).transpose(1, 0, 2, 3)
    ).reshape(128, -1).astype(BF16)
    Wwp = np.ascontiguousarray(
        np.asarray(W_w, dtype=np.float32).reshape(NC, 128).T
    ).astype(BF16)
    bSVp = np.ascontiguousarray(
        (np.asarray(b_S) + np.asarray(b_V)).astype(np.float32).reshape(NC, 128).T
    )
    bwp = np.full((N, 1), np.float32(np.asarray(b_w).reshape(-1)[0]))

    in_maps = []
    for c in range(NCORES):
        bidx = assign[c]
        hv_c = h_v[bidx]  # [BPC, T, D]
        hs_c = h_s[bidx]  # [BPC, N, D]
        # hvT: [p, kc, b, t] with d = kc*128+p
        hvT = np.ascontiguousarray(
            hv_c.transpose(2, 0, 1).reshape(NC, 128, BPC, T).transpose(1, 0, 2, 3)
        ).reshape(128, -1).astype(BF16)
        hsT = np.ascontiguousarray(
            hs_c.transpose(2, 0, 1).reshape(NC, 128, BPC, N).transpose(1, 0, 2, 3)
        ).reshape(128, -1).astype(BF16)
        hs_nat = np.ascontiguousarray(hs_c.transpose(1, 0, 2)).reshape(N, -1)
        mp = np.zeros((N, BPC), dtype=np.float32)
        for s in range(BPC):
            mp[: int(lengths[bidx[s]]), s] = 1.0
        in_maps.append(
            {
                "mp": mp,
                "hvT": hvT,
                "hsT": hsT,
                "hs": hs_nat,
                "WVp": WVp,
                "WSp": WSp,
                "Wwp": Wwp,
                "bSVp": bSVp,
                "bwp": bwp,
            }
        )
    return in_maps


def run(inputs: dict, trace: bool = False):
    """Run on 8 NeuronCores; returns (output, BassKernelResults)."""
    from concourse import bass_utils

    lengths = np.asarray(inputs["lengths"]).astype(np.int64)
    assign, n_slot = _plan(lengths)
    nc = _get_nc(n_slot)
    in_maps = _make_in_maps(
        inputs["h_s"], inputs["h_v"], lengths,
        inputs["W_S"], inputs["b_S"], inputs["W_V"], inputs["b_V"],
        inputs["W_w"], inputs["b_w"], assign,
    )
    res = bass_utils.run_bass_kernel_spmd(
        nc, in_maps, core_ids=list(range(NCORES)), trace=trace
    )
    h_s = np.asarray(inputs["h_s"], dtype=np.float32)
    full = np.empty((B, T, D), dtype=np.float32)
    for c in range(NCORES):
        out_c = res.results[c]["out"]
        for s in range(BPC):
            full[assign[c][s]] = out_c[s]
    # alpha's +1e-13 epsilon term, applied on host
    full += 1e-13 * h_s.sum(axis=1, keepdims=True)
    return full, res


def kernel(**inputs) -> np.ndarray:
    out, _ = run(inputs, trace=False)
    return out


# revision 13
# speedup vs baseline: 5.5982x; 1.1033x over previous
"""Trainium2 Bass kernel for InteractorwoLSTM additive attention.

out[b,t,:] = alpha[b,t,:] @ h_s[b]  with
  beta[b,t,n] = W_w . tanh(h_s[b,n]@W_S + b_S + h_v[b,t]@W_V + b_V) + b_w
  alpha = masked-softmax(beta) per reference semantics.

Sharding: data-parallel over batch B=32 across 8 cores (4 slots/core).

Key structural choices vs the naive version:
  - Masked n never contribute: q[n,t] = exp(beta*m)*m == 0 where m==0, and
    Z1's masked part is the constant count (30-L). So tanh/add/beta run
    only over n < L. The kernel is compiled per lengths-profile (cached);
    batches are bin-packed over cores so SPMD slot s has baked size
    N_slot[s] = max over cores of that slot's length.
  - All transposes (h_v^T, h_s^T) and tile packing happen on the host;
    every device DMA load is a single contiguous [P, X] transfer.
  - beta[n,t] = sum_d Ww[d]*e[d,n,t] via column-tiled matmuls: lhsT is the
    1-column Ww chunk at tile_position (0,32j), j=n%4, so 4 matmuls run
    concurrently in the PE array and beta lands multi-partition in PSUM.
  - Softmax collapses: Qs == Z1_mm (one ones-matmul), exp has no mask
    multiply, the alpha epsilon term (1e-13 * sum_n h_s) is added on host.
  - recip(denom) is folded into the final output copy as a per-partition
    tensor_scalar multiply.
"""

import ml_dtypes
import numpy as np

BF16 = ml_dtypes.bfloat16

B, T, N = 32, 128, 30
D = 512
NCORES = 8
BPC = B // NCORES  # batch slots per core
NC = D // 128  # 4 chunks of the feature dim

_CACHE = {}


def _build(n_slot: tuple[int, int, int, int]):
    import concourse.bacc as bacc
    import concourse.tile as tile
    from concourse import mybir

    f32 = mybir.dt.float32
    bf16 = mybir.dt.bfloat16

    nc = bacc.Bacc(
        "TRN2",
        target_bir_lowering=False,
        debug=False,
        enable_asserts=True,
        num_devices=NCORES,
    )

    # ---- DRAM I/O (host pre-packed, all contiguous) ----
    hvT_d = nc.dram_tensor("hvT", [128, NC * BPC * T], bf16, kind="ExternalInput").ap()
    hsT_d = nc.dram_tensor("hsT", [128, NC * BPC * N], bf16, kind="ExternalInput").ap()
    hs_d = nc.dram_tensor("hs", [N, BPC * D], f32, kind="ExternalInput").ap()
    WV_d = nc.dram_tensor("WVp", [128, NC * NC * 128], bf16, kind="ExternalInput").ap()
    WS_d = nc.dram_tensor("WSp", [128, NC * NC * 128], bf16, kind="ExternalInput").ap()
    Ww_d = nc.dram_tensor("Wwp", [128, NC], bf16, kind="ExternalInput").ap()
    bSV_d = nc.dram_tensor("bSVp", [128, NC], f32, kind="ExternalInput").ap()
    bw_d = nc.dram_tensor("bwp", [N, 1], f32, kind="ExternalInput").ap()
    mp_d = nc.dram_tensor("mp", [N, BPC], f32, kind="ExternalInput").ap()
    out_d = nc.dram_tensor("out", [BPC, T, D], f32, kind="ExternalOutput").ap()

    with tile.TileContext(nc) as tc:
        with (
            tc.tile_pool(name="const", bufs=1) as const,
            tc.tile_pool(name="proj", bufs=1) as projp,
            tc.tile_pool(name="epre", bufs=3) as eprep,
            tc.tile_pool(name="ebig", bufs=3) as ebigp,
            tc.tile_pool(name="soft", bufs=2) as softp,
            tc.tile_pool(name="pproj", bufs=1, space="PSUM") as pproj,
            tc.tile_pool(name="pbeta", bufs=2, space="PSUM") as pbeta,
            tc.tile_pool(name="pz", bufs=1, space="PSUM") as pzp,
            tc.tile_pool(name="pfin", bufs=2, space="PSUM") as pfin,
        ):
            # ---- constants / weights / activations (contiguous loads) ----
            WS_sb = const.tile([128, NC, NC, 128], bf16)
            nc.sync.dma_start(out=WS_sb[:], in_=WS_d)
            WV_sb = const.tile([128, NC, NC, 128], bf16)
            nc.scalar.dma_start(out=WV_sb[:], in_=WV_d)
            Ww_sb = const.tile([128, NC], bf16)
            nc.sync.dma_start(out=Ww_sb[:], in_=Ww_d)
            bSV_sb = const.tile([128, NC], f32)
            nc.sync.dma_start(out=bSV_sb[:], in_=bSV_d)
            bw_sb = const.tile([N, 1], f32)
            nc.sync.dma_start(out=bw_sb[:], in_=bw_d)
            mp_sb = const.tile([N, BPC], f32)
            nc.sync.dma_start(out=mp_sb[:], in_=mp_d)
            ones_sb = const.tile([N, 1], f32)
            nc.vector.memset(ones_sb[:], 1.0)
            z1_sb = const.tile([1, 128], f32)
            nc.vector.memset(z1_sb[:], 0.0)
            zrow_sb = const.tile([1, 512], f32)
            nc.vector.memset(zrow_sb[:], 0.0)

            hsT = const.tile([128, NC, BPC, N], bf16)
            nc.sync.dma_start(out=hsT[:], in_=hsT_d)
            hvT = const.tile([128, NC, BPC, T], bf16)
            hvT_v = hvT_d.rearrange("p (kc x) -> p kc x", kc=NC)
            for kc in range(NC):
                eng = nc.sync if kc % 2 == 0 else nc.scalar
                eng.dma_start(out=hvT[:, kc, :, :], in_=hvT_v[:, kc, :])
            hs_sb = const.tile([N, BPC, D], f32)
            nc.scalar.dma_start(out=hs_sb[:], in_=hs_d)

            # ---- projections, all batch slots at once, chunk-interleaved so
            # the first epre add only waits on chunk 0 ----
            VT = projp.tile([128, NC, BPC, T], bf16)
            ST = projp.tile([128, NC, BPC, N, 2], bf16)
            for mc in range(NC):
                ps_t = pproj.tile([128, BPC * T], f32, tag="pv", name="ps_t")
                ps = ps_t[:, : BPC * N]
                for kc in range(NC):
                    nc.tensor.matmul(
                        ps[:],
                        WS_sb[:, kc, mc, :],
                        hsT[:, kc, :, :],
                        start=(kc == 0),
                        stop=(kc == NC - 1),
                    )
                # fold b_S + b_V while copying out of PSUM; write each value
                # twice (pair-doubled layout) so the epre add's last AP dim is
                # step-1 x2 on every operand -> DVE 2x_1P mode
                nc.vector.tensor_scalar_add(
                    ST[:, mc, :, :, :],
                    ps[:].unsqueeze(2).broadcast_to([128, BPC * N, 2]).rearrange(
                        "p (b n) two -> p b n two", b=BPC
                    ),
                    bSV_sb[:, mc : mc + 1],
                )
                psv = pproj.tile([128, BPC * T], f32, tag="pv", name="psv")
                for kc in range(NC):
                    nc.tensor.matmul(
                        psv[:],
                        WV_sb[:, kc, mc, :],
                        hvT[:, kc, :, :],
                        start=(kc == 0),
                        stop=(kc == NC - 1),
                    )
                nc.vector.tensor_copy(VT[:, mc, :, :], psv[:])

            for b in range(BPC):
                L = n_slot[b]
                nr = (L + 3) // 4  # beta column-tile rounds
                # ---- e = tanh(VT (+bcast) ST), chunk by chunk; beta ----
                pbA = pbeta.tile([128, 512], f32, tag="bA", name="pbA")
                pbB = (
                    pbeta.tile([128, 512], f32, tag="bB", name="pbB")
                    if nr > 4
                    else None
                )
                # start=True clears the WHOLE 2KB bank's has-written bits, so
                # exactly one start per bank: a full-bank zeroing matmul whose
                # [128,512] write also forces WAW ordering before every
                # accumulate-only beta matmul below.
                for pb in (pbA, pbB):
                    if pb is not None:
                        nc.tensor.matmul(
                            pb[:],
                            z1_sb[:],
                            zrow_sb[:],
                            start=True,
                            stop=True,
                            skip_group_check=True,
                        )
                for c in range(NC):
                    epre_t = eprep.tile([128, N, T], bf16, tag="epre")
                    epre = epre_t[:, :L, :]
                    vt_b = (
                        VT[:, c, b, :]
                        .rearrange("p (a two) -> p a two", two=2)
                        .unsqueeze(1)
                        .broadcast_to([128, L, T // 2, 2])
                    )
                    st_b = (
                        ST[:, c, b, :L, :]
                        .unsqueeze(2)
                        .broadcast_to([128, L, T // 2, 2])
                    )
                    nc.vector.tensor_add(
                        epre.rearrange("p l (a two) -> p l a two", two=2),
                        vt_b,
                        st_b,
                    )
                    e_t = ebigp.tile([128, N, T], bf16, tag="e")
                    e_c = e_t[:, :L, :]
                    nc.scalar.activation(
                        e_c, epre, mybir.ActivationFunctionType.Tanh
                    )
                    # beta: per (bank, col-group j) one strided-rhs matmul
                    # accumulating Ww_c . e_c over r-slots; n = 4r + j.
                    for base, pb in ((0, pbA), (16, pbB)):
                        if pb is None:
                            continue
                        hi = min(L, base + 16)
                        for j in range(4):
                            nj = len(range(base + j, hi, 4))
                            if nj == 0:
                                continue
                            nc.tensor.matmul(
                                pb[32 * j : 32 * j + 1, 0 : nj * T],
                                Ww_sb[:, c : c + 1],
                                e_c[:, base + j : hi : 4, :],
                                start=False,
                                stop=False,
                                tile_position=(0, 32 * j),
                                skip_group_check=True,
                            )

                # ---- beta [4j, r, t] -> [n, t] via SBUF->SBUF DMA ----
                bsb = softp.tile([128, 8, T], f32, tag="bsb")
                nc.vector.tensor_copy(bsb[:, 0:4, :], pbA[:].rearrange("p (r t) -> p r t", r=4))
                if pbB is not None:
                    nc.vector.tensor_copy(
                        bsb[:, 4:8, :], pbB[:].rearrange("p (r t) -> p r t", r=4)
                    )
                beta_nt = softp.tile([N, T], f32, tag="bnt")
                for j in range(min(4, L)):
                    nj = (L - 1 - j) // 4 + 1  # count of n in {j, j+4, ...} < L
                    nc.sync.dma_start(
                        out=beta_nt[j : j + 4 * (nj - 1) + 1 : 4, :],
                        in_=bsb[32 * j : 32 * j + 1, 0:nj, :],
                    )

                # ---- collapsed softmax ----
                # t1 = exp(beta + b_w);  Z = sum_{n<L} t1 (ones-matmul)
                t1 = softp.tile([N, T], f32, tag="t1")
                nc.scalar.activation(
                    t1[:L, :],
                    beta_nt[:L, :],
                    mybir.ActivationFunctionType.Exp,
                    bias=bw_sb[:L, :],
                )
                # per-core mask within the slot: q = t1 * m  (m=0 for n>=L_b)
                q_t = softp.tile([N, T], f32, tag="q")
                nc.vector.tensor_scalar(
                    q_t[:L, :],
                    t1[:L, :],
                    mp_sb[:L, b : b + 1],
                    None,
                    op0=mybir.AluOpType.mult,
                )
                pz = pzp.tile([128, 1], f32, tag="pz")
                nc.tensor.matmul(
                    pz[:, 0:1], q_t[:L, :], ones_sb[:L, :], start=True, stop=True
                )
                recip = softp.tile([128, 1], f32, tag="recip")
                nc.vector.reciprocal(recip[:], pz[:, 0:1])

                # ---- out[t,:] = recip[t] * sum_{n<L} t1[n,t] * h_s[n,:] ----
                pf = pfin.tile([128, D], f32, tag="fin")
                nc.tensor.matmul(
                    pf[:], q_t[:L, :], hs_sb[:L, b, :], start=True, stop=True
                )
                out_sb = softp.tile([128, D], f32, tag="out")
                nc.vector.tensor_scalar(
                    out_sb[:],
                    pf[:],
                    recip[:],
                    None,
                    op0=mybir.AluOpType.mult,
                )
                nc.sync.dma_start(out=out_d[b], in_=out_sb[:])

    nc.compile()
    return nc


def _get_nc(n_slot):
    key = tuple(n_slot)
    if key not in _CACHE:
        _CACHE[key] = _build(key)
    return _CACHE[key]


def _plan(lengths):
    """Assign 32 batches to 8 cores x 4 slots, slot s holding the s-th
    longest of each core; returns (assign[core][slot] = batch idx, n_slot)."""
    order = np.argsort(-lengths, kind="stable")
    assign = [[int(order[s * NCORES + c]) for s in range(BPC)] for c in range(NCORES)]
    n_slot = []
    for s in range(BPC):
        m = max(int(lengths[assign[c][s]]) for c in range(NCORES))
        n_slot.append(m)
    return assign, tuple(n_slot)


def _make_in_maps(h_s, h_v, lengths, W_S, b_S, W_V, b_V, W_w, b_w, assign):
    h_s = np.ascontiguousarray(h_s, dtype=np.float32)
    h_v = np.ascontiguousarray(h_v, dtype=np.float32)
    WS = np.asarray(W_S, dtype=np.float32)
    WV = np.asarray(W_V, dtype=np.float32)
    # [k, d] -> [p, kc, mc, 128] with k = kc*128+p
    WSp = np.ascontiguousarray(
        WS.reshape(NC, 128, NC, 128).transpose(1, 0, 2, 3)
    ).reshape(128, -1).astype(BF16)
    WVp = np.ascontiguousarray(
        WV.reshape(NC, 128, NC, 128).transpose(1, 0, 2, 3)
    ).reshape(128, -1).astype(BF16)
    Wwp = np.ascontiguousarray(
        np.asarray(W_w, dtype=np.float32).reshape(NC, 128).T
    ).astype(BF16)
    bSVp = np.ascontiguousarray(
        (np.asarray(b_S) + np.asarray(b_V)).astype(np.float32).reshape(NC, 128).T
    )
    bwp = np.full((N, 1), np.float32(np.asarray(b_w).reshape(-1)[0]))

    in_maps = []
    for c in range(NCORES):
        bidx = assign[c]
        hv_c = h_v[bidx]  # [BPC, T, D]
        hs_c = h_s[bidx]  # [BPC, N, D]
        # hvT: [p, kc, b, t] with d = kc*128+p
        hvT = np.ascontiguousarray(
            hv_c.transpose(2, 0, 1).reshape(NC, 128, BPC, T).transpose(1, 0, 2, 3)
        ).reshape(128, -1).astype(BF16)
        hsT = np.ascontiguousarray(
            hs_c.transpose(2, 0, 1).reshape(NC, 128, BPC, N).transpose(1, 0, 2, 3)
        ).reshape(128, -1).astype(BF16)
        hs_nat = np.ascontiguousarray(hs_c.transpose(1, 0, 2)).reshape(N, -1)
        mp = np.zeros((N, BPC), dtype=np.float32)
        for s in range(BPC):
            mp[: int(lengths[bidx[s]]), s] = 1.0
        in_maps.append(
            {
                "mp": mp,
                "hvT": hvT,
                "hsT": hsT,
                "hs": hs_nat,
                "WVp": WVp,
                "WSp": WSp,
                "Wwp": Wwp,
                "bSVp": bSVp,
                "bwp": bwp,
            }
        )
    return in_maps


def run(inputs: dict, trace: bool = False):
    """Run on 8 NeuronCores; returns (output, BassKernelResults)."""
    from concourse import bass_utils

    lengths = np.asarray(inputs["lengths"]).astype(np.int64)
    assign, n_slot = _plan(lengths)
    nc = _get_nc(n_slot)
    in_maps = _make_in_maps(
        inputs["h_s"], inputs["h_v"], lengths,
        inputs["W_S"], inputs["b_S"], inputs["W_V"], inputs["b_V"],
        inputs["W_w"], inputs["b_w"], assign,
    )
    res = bass_utils.run_bass_kernel_spmd(
        nc, in_maps, core_ids=list(range(NCORES)), trace=trace
    )
    h_s = np.asarray(inputs["h_s"], dtype=np.float32)
    full = np.empty((B, T, D), dtype=np.float32)
    for c in range(NCORES):
        out_c = res.results[c]["out"]
        for s in range(BPC):
            full[assign[c][s]] = out_c[s]
    # alpha's +1e-13 epsilon term, applied on host
    full += 1e-13 * h_s.sum(axis=1, keepdims=True)
    return full, res


def kernel(**inputs) -> np.ndarray:
    out, _ = run(inputs, trace=False)
    return out


# revision 14
# speedup vs baseline: 5.6890x; 1.0162x over previous
"""Trainium2 Bass kernel for InteractorwoLSTM additive attention.

out[b,t,:] = alpha[b,t,:] @ h_s[b]  with
  beta[b,t,n] = W_w . tanh(h_s[b,n]@W_S + b_S + h_v[b,t]@W_V + b_V) + b_w
  alpha = masked-softmax(beta) per reference semantics.

Sharding: data-parallel over batch B=32 across 8 cores (4 slots/core).

Key structural choices vs the naive version:
  - Masked n never contribute: q[n,t] = exp(beta*m)*m == 0 where m==0, and
    Z1's masked part is the constant count (30-L). So tanh/add/beta run
    only over n < L. The kernel is compiled per lengths-profile (cached);
    batches are bin-packed over cores so SPMD slot s has baked size
    N_slot[s] = max over cores of that slot's length.
  - All transposes (h_v^T, h_s^T) and tile packing happen on the host;
    every device DMA load is a single contiguous [P, X] transfer.
  - beta[n,t] = sum_d Ww[d]*e[d,n,t] via column-tiled matmuls: lhsT is the
    1-column Ww chunk at tile_position (0,32j), j=n%4, so 4 matmuls run
    concurrently in the PE array and beta lands multi-partition in PSUM.
  - Softmax collapses: Qs == Z1_mm (one ones-matmul), exp has no mask
    multiply, the alpha epsilon term (1e-13 * sum_n h_s) is added on host.
  - recip(denom) is folded into the final output copy as a per-partition
    tensor_scalar multiply.
"""

import ml_dtypes
import numpy as np

BF16 = ml_dtypes.bfloat16

B, T, N = 32, 128, 30
D = 512
NCORES = 8
BPC = B // NCORES  # batch slots per core
NC = D // 128  # 4 chunks of the feature dim

_CACHE = {}


def _build(n_slot: tuple[int, int, int, int]):
    import concourse.bacc as bacc
    import concourse.tile as tile
    from concourse import mybir

    f32 = mybir.dt.float32
    bf16 = mybir.dt.bfloat16

    nc = bacc.Bacc(
        "TRN2",
        target_bir_lowering=False,
        debug=False,
        enable_asserts=True,
        num_devices=NCORES,
    )

    # ---- DRAM I/O (host pre-packed, all contiguous) ----
    hvT_d = nc.dram_tensor("hvT", [128, NC * BPC * T], bf16, kind="ExternalInput").ap()
    hsT_d = nc.dram_tensor("hsT", [128, NC * BPC * N], bf16, kind="ExternalInput").ap()
    hs_d = nc.dram_tensor("hs", [N, BPC * D], f32, kind="ExternalInput").ap()
    WV_d = nc.dram_tensor("WVp", [128, NC * NC * 128], bf16, kind="ExternalInput").ap()
    WS_d = nc.dram_tensor("WSp", [128, NC * NC * 128], bf16, kind="ExternalInput").ap()
    Ww_d = nc.dram_tensor("Wwp", [128, NC], bf16, kind="ExternalInput").ap()
    bSV_d = nc.dram_tensor("bSVp", [128, NC], f32, kind="ExternalInput").ap()
    bw_d = nc.dram_tensor("bwp", [N, 1], f32, kind="ExternalInput").ap()
    mp_d = nc.dram_tensor("mp", [N, BPC], f32, kind="ExternalInput").ap()
    out_d = nc.dram_tensor("out", [BPC, T, D], f32, kind="ExternalOutput").ap()

    with tile.TileContext(nc) as tc:
        with (
            tc.tile_pool(name="const", bufs=1) as const,
            tc.tile_pool(name="proj", bufs=1) as projp,
            tc.tile_pool(name="epre", bufs=3) as eprep,
            tc.tile_pool(name="ebig", bufs=3) as ebigp,
            tc.tile_pool(name="soft", bufs=2) as softp,
            tc.tile_pool(name="pproj", bufs=1, space="PSUM") as pproj,
            tc.tile_pool(name="pbeta", bufs=2, space="PSUM") as pbeta,
            tc.tile_pool(name="pz", bufs=1, space="PSUM") as pzp,
            tc.tile_pool(name="pfin", bufs=2, space="PSUM") as pfin,
        ):
            # ---- constants / weights / activations (contiguous loads) ----
            WS_sb = const.tile([128, NC, NC, 128], bf16)
            nc.sync.dma_start(out=WS_sb[:], in_=WS_d)
            WV_sb = const.tile([128, NC, NC, 128], bf16)
            nc.scalar.dma_start(out=WV_sb[:], in_=WV_d)
            ones_sb = const.tile([N, 1], f32)
            nc.vector.memset(ones_sb[:], 1.0)
            z1_sb = const.tile([1, 128], bf16)
            nc.vector.memset(z1_sb[:], 0.0)
            zrow_sb = const.tile([1, 512], bf16)
            nc.vector.memset(zrow_sb[:], 0.0)

            hsT = const.tile([128, NC, BPC, N], bf16)
            nc.sync.dma_start(out=hsT[:], in_=hsT_d)
            Ww_sb = const.tile([128, NC], bf16)
            nc.sync.dma_start(out=Ww_sb[:], in_=Ww_d)
            bSV_sb = const.tile([128, NC], f32)
            nc.sync.dma_start(out=bSV_sb[:], in_=bSV_d)
            bw_sb = const.tile([N, 1], f32)
            nc.sync.dma_start(out=bw_sb[:], in_=bw_d)
            mp_sb = const.tile([N, BPC], f32)
            nc.sync.dma_start(out=mp_sb[:], in_=mp_d)
            hvT = const.tile([128, NC, BPC, T], bf16)
            hvT_v = hvT_d.rearrange("p (kc x) -> p kc x", kc=NC)
            for kc in range(NC):
                eng = nc.sync if kc % 2 == 0 else nc.scalar
                eng.dma_start(out=hvT[:, kc, :, :], in_=hvT_v[:, kc, :])
            hs_sb = const.tile([N, BPC, D], f32)
            nc.scalar.dma_start(out=hs_sb[:], in_=hs_d)

            # ---- projections, all batch slots at once, chunk-interleaved so
            # the first epre add only waits on chunk 0 ----
            VT = projp.tile([128, NC, BPC, T], bf16)
            ST = projp.tile([128, NC, BPC, N, 2], bf16)
            for mc in range(NC):
                ps_t = pproj.tile([128, BPC * T], f32, tag="pv", name="ps_t")
                ps = ps_t[:, : BPC * N]
                for kc in range(NC):
                    nc.tensor.matmul(
                        ps[:],
                        WS_sb[:, kc, mc, :],
                        hsT[:, kc, :, :],
                        start=(kc == 0),
                        stop=(kc == NC - 1),
                    )
                # fold b_S + b_V while copying out of PSUM; write each value
                # twice (pair-doubled layout) so the epre add's last AP dim is
                # step-1 x2 on every operand -> DVE 2x_1P mode
                nc.vector.tensor_scalar_add(
                    ST[:, mc, :, :, :],
                    ps[:].unsqueeze(2).broadcast_to([128, BPC * N, 2]).rearrange(
                        "p (b n) two -> p b n two", b=BPC
                    ),
                    bSV_sb[:, mc : mc + 1],
                )
                psv = pproj.tile([128, BPC * T], f32, tag="pv", name="psv")
                for kc in range(NC):
                    nc.tensor.matmul(
                        psv[:],
                        WV_sb[:, kc, mc, :],
                        hvT[:, kc, :, :],
                        start=(kc == 0),
                        stop=(kc == NC - 1),
                    )
                nc.vector.tensor_copy(VT[:, mc, :, :], psv[:])

            for b in range(BPC):
                L = n_slot[b]
                nr = (L + 3) // 4  # beta column-tile rounds
                # ---- e = tanh(VT (+bcast) ST), chunk by chunk; beta ----
                pbA = pbeta.tile([128, 512], f32, tag="bA", name="pbA")
                pbB = (
                    pbeta.tile([128, 512], f32, tag="bB", name="pbB")
                    if nr > 4
                    else None
                )
                # start=True clears the WHOLE 2KB bank's has-written bits, so
                # exactly one start per bank: a full-bank zeroing matmul whose
                # [128,512] write also forces WAW ordering before every
                # accumulate-only beta matmul below.
                for pb in (pbA, pbB):
                    if pb is not None:
                        nc.tensor.matmul(
                            pb[:],
                            z1_sb[:],
                            zrow_sb[:],
                            start=True,
                            stop=True,
                            skip_group_check=True,
                        )
                for c in range(NC):
                    epre_t = eprep.tile([128, N, T], bf16, tag="epre")
                    epre = epre_t[:, :L, :]
                    vt_b = (
                        VT[:, c, b, :]
                        .rearrange("p (a two) -> p a two", two=2)
                        .unsqueeze(1)
                        .broadcast_to([128, L, T // 2, 2])
                    )
                    st_b = (
                        ST[:, c, b, :L, :]
                        .unsqueeze(2)
                        .broadcast_to([128, L, T // 2, 2])
                    )
                    nc.vector.tensor_add(
                        epre.rearrange("p l (a two) -> p l a two", two=2),
                        vt_b,
                        st_b,
                    )
                    e_t = ebigp.tile([128, N, T], bf16, tag="e")
                    e_c = e_t[:, :L, :]
                    nc.scalar.activation(
                        e_c, epre, mybir.ActivationFunctionType.Tanh
                    )
                    # beta: per (bank, col-group j) one strided-rhs matmul
                    # accumulating Ww_c . e_c over r-slots; n = 4r + j.
                    for base, pb in ((0, pbA), (16, pbB)):
                        if pb is None:
                            continue
                        hi = min(L, base + 16)
                        for j in range(4):
                            nj = len(range(base + j, hi, 4))
                            if nj == 0:
                                continue
                            nc.tensor.matmul(
                                pb[32 * j : 32 * j + 1, 0 : nj * T],
                                Ww_sb[:, c : c + 1],
                                e_c[:, base + j : hi : 4, :],
                                start=False,
                                stop=False,
                                tile_position=(0, 32 * j),
                                skip_group_check=True,
                            )

                # ---- beta [4j, r, t] -> [n, t] via SBUF->SBUF DMA ----
                bsb = softp.tile([128, 8, T], f32, tag="bsb")
                nc.vector.tensor_copy(bsb[:, 0:4, :], pbA[:].rearrange("p (r t) -> p r t", r=4))
                if pbB is not None:
                    nc.vector.tensor_copy(
                        bsb[:, 4:8, :], pbB[:].rearrange("p (r t) -> p r t", r=4)
                    )
                beta_nt = softp.tile([N, T], f32, tag="bnt")
                for j in range(min(4, L)):
                    nj = (L - 1 - j) // 4 + 1  # count of n in {j, j+4, ...} < L
                    nc.sync.dma_start(
                        out=beta_nt[j : j + 4 * (nj - 1) + 1 : 4, :],
                        in_=bsb[32 * j : 32 * j + 1, 0:nj, :],
                    )

                # ---- collapsed softmax ----
                # t1 = exp(beta + b_w);  Z = sum_{n<L} t1 (ones-matmul)
                t1 = softp.tile([N, T], f32, tag="t1")
                nc.scalar.activation(
                    t1[:L, :],
                    beta_nt[:L, :],
                    mybir.ActivationFunctionType.Exp,
                    bias=bw_sb[:L, :],
                )
                # per-core mask within the slot: q = t1 * m  (m=0 for n>=L_b)
                q_t = softp.tile([N, T], f32, tag="q")
                nc.vector.tensor_scalar(
                    q_t[:L, :],
                    t1[:L, :],
                    mp_sb[:L, b : b + 1],
                    None,
                    op0=mybir.AluOpType.mult,
                )
                pz = pzp.tile([128, 1], f32, tag="pz")
                nc.tensor.matmul(
                    pz[:, 0:1], q_t[:L, :], ones_sb[:L, :], start=True, stop=True
                )
                recip = softp.tile([128, 1], f32, tag="recip")
                nc.vector.reciprocal(recip[:], pz[:, 0:1])

                # ---- out[t,:] = recip[t] * sum_{n<L} t1[n,t] * h_s[n,:] ----
                pf = pfin.tile([128, D], f32, tag="fin")
                nc.tensor.matmul(
                    pf[:], q_t[:L, :], hs_sb[:L, b, :], start=True, stop=True
                )
                out_sb = softp.tile([128, D], f32, tag="out")
                nc.vector.tensor_scalar(
                    out_sb[:],
                    pf[:],
                    recip[:],
                    None,
                    op0=mybir.AluOpType.mult,
                )
                nc.sync.dma_start(out=out_d[b], in_=out_sb[:])

    nc.compile()
    return nc


def _get_nc(n_slot):
    key = tuple(n_slot)
    if key not in _CACHE:
        _CACHE[key] = _build(key)
    return _CACHE[key]


def _plan(lengths):
    """Assign 32 batches to 8 cores x 4 slots, slot s holding the s-th
    longest of each core; returns (assign[core][slot] = batch idx, n_slot)."""
    order = np.argsort(-lengths, kind="stable")
    assign = [[int(order[s * NCORES + c]) for s in range(BPC)] for c in range(NCORES)]
    n_slot = []
    for s in range(BPC):
        m = max(int(lengths[assign[c][s]]) for c in range(NCORES))
        n_slot.append(m)
    return assign, tuple(n_slot)


def _make_in_maps(h_s, h_v, lengths, W_S, b_S, W_V, b_V, W_w, b_w, assign):
    h_s = np.ascontiguousarray(h_s, dtype=np.float32)
    h_v = np.ascontiguousarray(h_v, dtype=np.float32)
    WS = np.asarray(W_S, dtype=np.float32)
    WV = np.asarray(W_V, dtype=np.float32)
    # [k, d] -> [p, kc, mc, 128] with k = kc*128+p
    WSp = np.ascontiguousarray(
        WS.reshape(NC, 128, NC, 128).transpose(1, 0, 2, 3)
    ).reshape(128, -1).astype(BF16)
    WVp = np.ascontiguousarray(
        WV.reshape(NC, 128, NC, 128).transpose(1, 0, 2, 3)
    ).reshape(128, -1).astype(BF16)
    Wwp = np.ascontiguousarray(
        np.asarray(W_w, dtype=np.float32).reshape(NC, 128).T
    ).astype(BF16)
    bSVp = np.ascontiguousarray(
        (np.asarray(b_S) + np.asarray(b_V)).astype(np.float32).reshape(NC, 128).T
    )
    bwp = np.full((N, 1), np.float32(np.asarray(b_w).reshape(-1)[0]))

    in_maps = []
    for c in range(NCORES):
        bidx = assign[c]
        hv_c = h_v[bidx]  # [BPC, T, D]
        hs_c = h_s[bidx]  # [BPC, N, D]
        # hvT: [p, kc, b, t] with d = kc*128+p
        hvT = np.ascontiguousarray(
            hv_c.transpose(2, 0, 1).reshape(NC, 128, BPC, T).transpose(1, 0, 2, 3)
        ).reshape(128, -1).astype(BF16)
        hsT = np.ascontiguousarray(
            hs_c.transpose(2, 0, 1).reshape(NC, 128, BPC, N).transpose(1, 0, 2, 3)
        ).reshape(128, -1).astype(BF16)
        hs_nat = np.ascontiguousarray(hs_c.transpose(1, 0, 2)).reshape(N, -1)
        mp = np.zeros((N, BPC), dtype=np.float32)
        for s in range(BPC):
            mp[: int(lengths[bidx[s]]), s] = 1.0
        in_maps.append(
            {
                "mp": mp,
                "hvT": hvT,
                "hsT": hsT,
                "hs": hs_nat,
                "WVp": WVp,
                "WSp": WSp,
                "Wwp": Wwp,
                "bSVp": bSVp,
                "bwp": bwp,
            }
        )
    return in_maps


def run(inputs: dict, trace: bool = False):
    """Run on 8 NeuronCores; returns (output, BassKernelResults)."""
    from concourse import bass_utils

    lengths = np.asarray(inputs["lengths"]).astype(np.int64)
    assign, n_slot = _plan(lengths)
    nc = _get_nc(n_slot)
    in_maps = _make_in_maps(
        inputs["h_s"], inputs["h_v"], lengths,
        inputs["W_S"], inputs["b_S"], inputs["W_V"], inputs["b_V"],
        inputs["W_w"], inputs["b_w"], assign,
    )
    res = bass_utils.run_bass_kernel_spmd(
        nc, in_maps, core_ids=list(range(NCORES)), trace=trace
    )
    h_s = np.asarray(inputs["h_s"], dtype=np.float32)
    full = np.empty((B, T, D), dtype=np.float32)
    for c in range(NCORES):
        out_c = res.results[c]["out"]
        for s in range(BPC):
            full[assign[c][s]] = out_c[s]
    # alpha's +1e-13 epsilon term, applied on host
    full += 1e-13 * h_s.sum(axis=1, keepdims=True)
    return full, res


def kernel(**inputs) -> np.ndarray:
    out, _ = run(inputs, trace=False)
    return out
